# revision 26
# baseline (speedup 1.0000x reference)
"""Masked-BCE mean loss kernel for Trainium2, data-parallel over 8 NeuronCores.

Math (targets t are exactly 0.0/1.0):
    bce(x, t) = softplus(x) - x*t = softplus((1-2t)*x)
    row mask  = 1[t0 + t1 > 0]
    answer    = sum(mask * (bce0 + bce1)) / (B*C)

Per-core plan (shard = 2^21 elements, 8 tiles of [128 x 2048] f32):
    DVE : W = 1 - 2T          (tensor_scalar, 2x mode)
          Y = W * X           (tensor_tensor)
          U = T_even + T_odd  (pair sum via strided APs, half-width)
          M = (U > 0)         (tensor_scalar, 2x mode)
    ACT : E = exp(Y); S = ln(E + 1)   (softplus; exp+ln share one table set)
    PE  : psum[m, n] += sum_p M[p, m] * S[p, n]  per 128x256 chunk; the
          generalized-diagonal stripes (m, 2m), (m, 2m+1) of the accumulated
          [128, 256] PSUM hold the masked-bce partial sums; everything else
          is ignored.
Host: sum stripes over the 8 per-core outputs in f64, divide by B*C.
"""

import sys

import numpy as np

for _p in ("/opt/trn_rl_repo",):
    if _p not in sys.path:
        sys.path.insert(0, _p)

import concourse.bass as bass  # noqa: E402
import concourse.tile as tile  # noqa: E402
from concourse import bacc, mybir  # noqa: E402
from concourse.bass_utils import run_bass_kernel_spmd  # noqa: E402

N_CORES = 8
B = 8388608
C = 2
SHARD = B * C // N_CORES  # 2097152 f32 elements per core
P = 128
F = 2048  # free-dim elements per partition per tile
TILE_ELEMS = P * F
N_TILES = SHARD // TILE_ELEMS  # 8

dt = mybir.dt
AF = mybir.ActivationFunctionType
ALU = mybir.AluOpType

_CACHE: dict[str, object] = {}


def _patch_act_tables():
    """Make Exp and Ln resolve to the single covering table set.

    The act-table placement pass picks, per activation, some set containing
    the needed function; with Exp and Ln alternating per tile it ping-pongs
    between `exp_and_others` and `natural_log` (one ~2.7us ACT_TABLE_LOAD per
    tile).  Hiding Exp/Ln from every other set (preserving list order, so
    `act_func_set_id` indices stay aligned with act_info.json) forces both
    onto `natural_log_exp_and_others` -> a single load for the whole kernel.
    """
    if _CACHE.get("act_patched"):
        return
    import concourse.hw_specs as hw_specs

    orig = hw_specs.get_activation_tables

    def patched(module_arch):
        tabs = orig(module_arch)
        out = {}
        for name, funcs in tabs.items():
            if name == "natural_log_exp_and_others":
                out[name] = set(funcs)
            else:
                out[name] = set(funcs) - {AF.Exp, AF.Ln}
        return out

    bacc.get_activation_tables = patched
    _CACHE["act_patched"] = True


def _build_nc():
    _patch_act_tables()
    nc = bacc.Bacc(
        "TRN2", target_bir_lowering=False, debug=False, num_devices=N_CORES
    )
    x_d = nc.dram_tensor("x", [SHARD], dt.bfloat16, kind="ExternalInput").ap()
    t_d = nc.dram_tensor("t", [SHARD], dt.bfloat16, kind="ExternalInput").ap()
    out_d = nc.dram_tensor("out", [P, 256], dt.float32, kind="ExternalOutput").ap()

    x_f = x_d.rearrange("(n f) -> n f", f=F)  # [P*N_TILES, F]
    t_f = t_d.rearrange("(n f) -> n f", f=F)

    # chunk schedule: full tiles, with the last tile split in half so the
    # tail drain (W->Y->exp->ln->matmuls) after the final DMA is shorter
    chunks = []  # (row_start, f) with row ranges in units of F columns
    row = 0
    for i in range(N_TILES - 1):
        chunks.append((row, F))
        row += P
    chunks.append((row, F // 2))
    chunks.append((row, F // 2))

    with tile.TileContext(nc) as tc:
        with (
            tc.tile_pool(name="io", bufs=4) as io_pool,
            tc.tile_pool(name="work", bufs=3) as work_pool,
            tc.tile_pool(name="acc", bufs=1, space="PSUM") as psum_pool,
            tc.tile_pool(name="outp", bufs=1) as out_pool,
        ):
            # batch pairs of full tiles through one exp/ln pass each; the
            # trailing full tile + two half tiles stay solo for a short tail
            batches = [[0, 1], [2, 3], [4, 5], [6], [7], [8]]
            acc = psum_pool.tile([P, 256], dt.float32)
            half_done = False
            n_mm = 0
            total_mm = sum(f // 256 for _, f in chunks)
            for batch in batches:
                fs = [chunks[ci][1] for ci in batch]
                fb = sum(fs)
                Y = work_pool.tile([P, fb], dt.bfloat16, tag="Y")
                Ms = []
                offs = []
                off = 0
                for ci in batch:
                    row0, f = chunks[ci]
                    col0 = 0
                    if f != F:
                        col0 = F // 2 if half_done else 0
                        half_done = True
                    x_src = x_f[row0 : row0 + P, col0 : col0 + f]
                    t_src = t_f[row0 : row0 + P, col0 : col0 + f]

                    T = io_pool.tile([P, f], dt.bfloat16, tag="T")
                    nc.sync.dma_start(T[:], t_src)
                    X = io_pool.tile([P, f], dt.bfloat16, tag="X")
                    nc.sync.dma_start(X[:], x_src)

                    W = work_pool.tile([P, f], dt.bfloat16, tag="W")
                    nc.vector.tensor_scalar(
                        W[:], T[:], -2.0, 1.0, ALU.mult, ALU.add
                    )
                    # all-bf16 unit-stride tensor_tensor -> DVE 2x mode;
                    # y = +-x stays exact because w is +-1
                    nc.vector.tensor_tensor(
                        Y[:, off : off + f], W[:], X[:], ALU.mult
                    )

                    Tp = T[:].rearrange("p (n two) -> p n two", two=2)
                    M = work_pool.tile([P, f // 2], dt.bfloat16, tag="M")
                    nc.vector.tensor_tensor(
                        M[:], Tp[:, :, 0], Tp[:, :, 1], ALU.logical_or
                    )
                    Ms.append(M)
                    offs.append((off, f))
                    off += f

                E = work_pool.tile([P, fb], dt.float32, tag="E")
                nc.scalar.activation(E[:], Y[:], AF.Exp)
                S = work_pool.tile([P, fb], dt.bfloat16, tag="S")
                nc.scalar.activation(S[:], E[:], AF.Ln, bias=1.0)

                for M, (off, f) in zip(Ms, offs):
                    for ch in range(f // 256):
                        nc.tensor.matmul(
                            acc[:],
                            lhsT=M[:, ch * 128 : (ch + 1) * 128],
                            rhs=S[:, off + ch * 256 : off + (ch + 1) * 256],
                            start=(n_mm == 0),
                            stop=(n_mm == total_mm - 1),
                        )
                        n_mm += 1

            out_s = out_pool.tile([P, 256], dt.float32)
            nc.scalar.copy(out_s[:], acc[:])
            nc.sync.dma_start(out_d[:], out_s[:])

    nc.compile()
    return nc


def _get_nc():
    if "nc" not in _CACHE:
        _CACHE["nc"] = _build_nc()
    return _CACHE["nc"]


def _reduce_outputs(outs: list[np.ndarray]) -> np.ndarray:
    j = np.arange(P)
    total = 0.0
    for o in outs:
        o64 = o.astype(np.float64)
        total += o64[j, 2 * j].sum() + o64[j, 2 * j + 1].sum()
    return np.asarray(total / (B * C), dtype=np.float32)


def make_in_maps(inputs: np.ndarray, targets: np.ndarray) -> list[dict]:
    import ml_dtypes

    # x in bf16: the only error is the unbiased per-element rounding of x,
    # which averages out over the 2^24-element mean (measured ~6e-7 rel).
    # t in bf16 is lossless (exactly 0.0/1.0). Halves DMA traffic for both.
    xs = (
        np.ascontiguousarray(inputs, dtype=np.float32)
        .astype(ml_dtypes.bfloat16)
        .reshape(N_CORES, SHARD)
    )
    ts = (
        np.ascontiguousarray(targets, dtype=np.float32)
        .astype(ml_dtypes.bfloat16)
        .reshape(N_CORES, SHARD)
    )
    return [{"x": xs[c], "t": ts[c]} for c in range(N_CORES)]


def kernel(inputs: np.ndarray, targets: np.ndarray) -> np.ndarray:
    nc = _get_nc()
    in_maps = make_in_maps(inputs, targets)
    res = run_bass_kernel_spmd(nc, in_maps, list(range(N_CORES)))
    outs = [res.results[c]["out"] for c in range(N_CORES)]
    return _reduce_outputs(outs)


# revision 29
# speedup vs baseline: 1.0555x; 1.0555x over previous
"""Masked-BCE mean loss kernel for Trainium2, data-parallel over 8 NeuronCores.

Math (targets t are exactly 0.0/1.0):
    bce(x, t) = softplus(x) - x*t = softplus((1-2t)*x)
    row mask  = 1[t0 + t1 > 0]
    answer    = sum(mask * (bce0 + bce1)) / (B*C)

Per-core plan (shard = 2^21 elements, 8 tiles of [128 x 2048] f32):
    DVE : W = 1 - 2T          (tensor_scalar, 2x mode)
          Y = W * X           (tensor_tensor)
          U = T_even + T_odd  (pair sum via strided APs, half-width)
          M = (U > 0)         (tensor_scalar, 2x mode)
    ACT : E = exp(Y); S = ln(E + 1)   (softplus; exp+ln share one table set)
    PE  : psum[m, n] += sum_p M[p, m] * S[p, n]  per 128x256 chunk; the
          generalized-diagonal stripes (m, 2m), (m, 2m+1) of the accumulated
          [128, 256] PSUM hold the masked-bce partial sums; everything else
          is ignored.
Host: sum stripes over the 8 per-core outputs in f64, divide by B*C.
"""

import sys

import numpy as np

for _p in ("/opt/trn_rl_repo",):
    if _p not in sys.path:
        sys.path.insert(0, _p)

import concourse.bass as bass  # noqa: E402
import concourse.tile as tile  # noqa: E402
from concourse import bacc, mybir  # noqa: E402
from concourse.bass_utils import run_bass_kernel_spmd  # noqa: E402

N_CORES = 8
B = 8388608
C = 2
SHARD = B * C // N_CORES  # 2097152 f32 elements per core
P = 128
F = 2048  # free-dim elements per partition per tile
TILE_ELEMS = P * F
N_TILES = SHARD // TILE_ELEMS  # 8

dt = mybir.dt
AF = mybir.ActivationFunctionType
ALU = mybir.AluOpType

_CACHE: dict[str, object] = {}


def _patch_act_tables():
    """Make Exp and Ln resolve to the single covering table set.

    The act-table placement pass picks, per activation, some set containing
    the needed function; with Exp and Ln alternating per tile it ping-pongs
    between `exp_and_others` and `natural_log` (one ~2.7us ACT_TABLE_LOAD per
    tile).  Hiding Exp/Ln from every other set (preserving list order, so
    `act_func_set_id` indices stay aligned with act_info.json) forces both
    onto `natural_log_exp_and_others` -> a single load for the whole kernel.
    """
    if _CACHE.get("act_patched"):
        return
    import concourse.hw_specs as hw_specs

    orig = hw_specs.get_activation_tables

    def patched(module_arch):
        tabs = orig(module_arch)
        out = {}
        for name, funcs in tabs.items():
            if name == "natural_log_exp_and_others":
                out[name] = set(funcs)
            else:
                out[name] = set(funcs) - {AF.Exp, AF.Ln}
        return out

    bacc.get_activation_tables = patched
    _CACHE["act_patched"] = True


def _build_nc():
    _patch_act_tables()
    nc = bacc.Bacc(
        "TRN2", target_bir_lowering=False, debug=False, num_devices=N_CORES
    )
    x_d = nc.dram_tensor("x", [SHARD], dt.bfloat16, kind="ExternalInput").ap()
    t_d = nc.dram_tensor("t", [SHARD], dt.bfloat16, kind="ExternalInput").ap()
    out_d = nc.dram_tensor("out", [P, 256], dt.float32, kind="ExternalOutput").ap()

    x_f = x_d.rearrange("(n f) -> n f", f=F)  # [P*N_TILES, F]
    t_f = t_d.rearrange("(n f) -> n f", f=F)

    # chunk schedule: full tiles, with the first tile split (prime the
    # ACT pipeline sooner) and the last tile split (shorter tail drain)
    chunks = [(0, 0, F // 2), (0, F // 2, F // 2)]  # (row0, col0, f)
    row = P
    for i in range(N_TILES - 2):
        chunks.append((row, 0, F))
        row += P
    chunks.append((row, 0, F // 2))
    chunks.append((row, F // 2, F // 2))

    with tile.TileContext(nc) as tc:
        with (
            tc.tile_pool(name="io", bufs=4) as io_pool,
            tc.tile_pool(name="work", bufs=3) as work_pool,
            tc.tile_pool(name="acc", bufs=1, space="PSUM") as psum_pool,
            tc.tile_pool(name="outp", bufs=1) as out_pool,
        ):
            acc = psum_pool.tile([P, 256], dt.float32)
            n_mm = 0
            total_mm = sum(f // 256 for _, _, f in chunks)
            for row0, col0, f in chunks:
                x_src = x_f[row0 : row0 + P, col0 : col0 + f]
                t_src = t_f[row0 : row0 + P, col0 : col0 + f]

                T = io_pool.tile([P, f], dt.bfloat16, tag="T")
                nc.sync.dma_start(T[:], t_src)
                X = io_pool.tile([P, f], dt.bfloat16, tag="X")
                nc.sync.dma_start(X[:], x_src)

                W = work_pool.tile([P, f], dt.bfloat16, tag="W")
                nc.vector.tensor_scalar(W[:], T[:], -2.0, 1.0, ALU.mult, ALU.add)
                # all-bf16 unit-stride tensor_tensor -> DVE 2x mode; y = +-x
                # stays exact because w is +-1
                Y = work_pool.tile([P, f], dt.bfloat16, tag="Y")
                nc.vector.tensor_tensor(Y[:], W[:], X[:], ALU.mult)

                Tp = T[:].rearrange("p (n two) -> p n two", two=2)
                M = work_pool.tile([P, f // 2], dt.bfloat16, tag="M")
                nc.vector.tensor_tensor(M[:], Tp[:, :, 0], Tp[:, :, 1], ALU.logical_or)

                E = work_pool.tile([P, f], dt.float32, tag="E")
                nc.scalar.activation(E[:], Y[:], AF.Exp)
                S = work_pool.tile([P, f], dt.bfloat16, tag="S")
                nc.scalar.activation(S[:], E[:], AF.Ln, bias=1.0)

                for ch in range(f // 256):
                    nc.tensor.matmul(
                        acc[:],
                        lhsT=M[:, ch * 128 : (ch + 1) * 128],
                        rhs=S[:, ch * 256 : (ch + 1) * 256],
                        start=(n_mm == 0),
                        stop=(n_mm == total_mm - 1),
                    )
                    n_mm += 1

            out_s = out_pool.tile([P, 256], dt.float32)
            nc.scalar.copy(out_s[:], acc[:])
            nc.sync.dma_start(out_d[:], out_s[:])

    nc.compile()
    return nc


def _get_nc():
    if "nc" not in _CACHE:
        _CACHE["nc"] = _build_nc()
    return _CACHE["nc"]


def _reduce_outputs(outs: list[np.ndarray]) -> np.ndarray:
    j = np.arange(P)
    total = 0.0
    for o in outs:
        o64 = o.astype(np.float64)
        total += o64[j, 2 * j].sum() + o64[j, 2 * j + 1].sum()
    return np.asarray(total / (B * C), dtype=np.float32)


def make_in_maps(inputs: np.ndarray, targets: np.ndarray) -> list[dict]:
    import ml_dtypes

    # x in bf16: the only error is the unbiased per-element rounding of x,
    # which averages out over the 2^24-element mean (measured ~6e-7 rel).
    # t in bf16 is lossless (exactly 0.0/1.0). Halves DMA traffic for both.
    xs = (
        np.ascontiguousarray(inputs, dtype=np.float32)
        .astype(ml_dtypes.bfloat16)
        .reshape(N_CORES, SHARD)
    )
    ts = (
        np.ascontiguousarray(targets, dtype=np.float32)
        .astype(ml_dtypes.bfloat16)
        .reshape(N_CORES, SHARD)
    )
    return [{"x": xs[c], "t": ts[c]} for c in range(N_CORES)]


def kernel(inputs: np.ndarray, targets: np.ndarray) -> np.ndarray:
    nc = _get_nc()
    in_maps = make_in_maps(inputs, targets)
    res = run_bass_kernel_spmd(nc, in_maps, list(range(N_CORES)))
    outs = [res.results[c]["out"] for c in range(N_CORES)]
    return _reduce_outputs(outs)
